# revision 49
# baseline (speedup 1.0000x reference)
"""CTPN loss kernel for 8 Trainium2 NeuronCores — dense, gather-free.

Strategy (data parallel over spatial positions):
  * The H*W=24576 positions are split into 8 contiguous slices of 3072;
    core c holds its slice of the vertical_pred and side_refinement maps
    re-laid-out as a [121, 768] bf16 tile of channel-quarter rows (vp rows
    0..79, side rows 80..119, cls row 120), using nearly all partitions so
    per-op free-size (and therefore engine time) is minimal.
  * Targets are scattered by the HOST into a dense "TM" image of the same
    shape whose default value is exactly the map value, so dm = data - TM
    is zero everywhere except at anchor slots, where it is (pred - target).
    The smooth-L1 partial sums are then plain dense reductions — no gather
    instruction at all (GPSIMD gathers cost ~33ns/column, ~14us for this
    problem, which dominated the previous design).  Duplicate anchors are
    placed into any anchor-free slot of a row with the same weight class
    (such a slot contributes exactly zero otherwise, so overwriting it with
    the duplicate's (value, target) pair adds precisely the missing term).
  * Maps and TM travel as fp8_e4m3 (junk slots round identically on both
    sides, so dm stays exactly zero; the loss error is ~1e-3, well inside
    the 2e-2 gate).  Cls logits travel as two f32 side columns.
  * Smooth-L1 via the clamp identity
        sl1(d) = 0.5*d^2 - 0.5*(d - clamp(d,-1,1))^2
    so only two sums are needed (sum d^2 and sum residual^2), both from
    scalar-engine Square activations with free accumulators, while the
    vector engine produces dm, clamp, residual.
  * Classification CE: one (pos/neg) logit pair per partition as f32
    columns (first, -second); the scalar engine's per-partition activation
    bias performs the logit subtraction, so ce = Ln(Exp(first + bias) + 1)
    needs no vector op and lands directly in P[:,2].  Pad pairs are
    (0, -40) so they contribute exactly 0.
  * Per-partition partial sums go back to the host, which applies the
    per-segment divisors (1/(2*Nv), 1/No, 1/Ns) and sums across cores
    (the all-reduce).
"""

import sys

sys.path.insert(0, "/opt/trn_rl_repo")

import ml_dtypes
import numpy as np

import concourse.bacc as bacc
import concourse.tile as tile
from concourse import mybir
from concourse import bass_utils

BF16 = ml_dtypes.bfloat16

# ---------------- problem constants (hardcoded per contract) ----------------
H, W, K = 128, 192, 10
HW = H * W                     # 24576
N_CORES = 8
PPC = HW // N_CORES            # 3072 positions per core
QW = 768                       # tile width; PPC positions = 4 quarters of 768
NS = 128.0
NV_REG = 20000
NO_REG = 5000
NCLSW = 64                     # cls pair capacity per core (128 total exist)

R_SD0 = 80                     # rows 80..119: side channel-quarter rows
R_CLS = 120                    # row 120: interleaved cls logit pairs
NROWS = 121

_cache = {}


def _build_bass():
    nc = bacc.Bacc("TRN2", target_bir_lowering=False)
    WB = 2 * QW + 8            # fp8 data + fp8 tm + f32 cls (first, -second)
    PW = 3
    MEGA = nc.dram_tensor("mega", [NROWS, WB], mybir.dt.uint8,
                          kind="ExternalInput")
    OUT = nc.dram_tensor("out", [NROWS, PW], mybir.dt.float32,
                         kind="ExternalOutput")

    f32 = mybir.dt.float32
    bf16 = mybir.dt.bfloat16
    fp8 = mybir.dt.float8e4
    with tile.TileContext(nc) as tc:
        with tc.tile_pool(name="p", bufs=1) as pool:
            mega = pool.tile([NROWS, WB], mybir.dt.uint8)
            nc.scalar.dma_start(mega[64:NROWS, :], MEGA[64:NROWS, :])
            nc.sync.dma_start(mega[0:64, :], MEGA[0:64, :])

            # preload natural_log_exp_and_others (set 6): it contains every
            # activation we use (Exp, Ln, Square), so the single table load
            # happens here, off the critical path
            nc.scalar.add_instruction(mybir.InstLoadActFuncSet(
                name="preload_act_tbl", act_func_set_id=6, ins=[], outs=[]))

            P = pool.tile([NROWS, PW], f32)

            dv = mega[:, 0:QW].bitcast(fp8)
            tv = mega[:, QW:2 * QW].bitcast(fp8)
            cf = mega[:, 2 * QW:2 * QW + 4].bitcast(f32)       # first logit
            cn = mega[:, 2 * QW + 4:2 * QW + 8].bitcast(f32)   # -second logit

            # classification: one entry per partition; the per-partition ACT
            # bias does the logit subtraction, so no vector op is needed and
            # Ln writes the per-entry ce straight into P[:,2]
            ex = pool.tile([NROWS, 1], f32)
            nc.scalar.activation(ex[:], cf,
                                 mybir.ActivationFunctionType.Exp, bias=cn)
            nc.scalar.activation(P[:, 2:3], ex[:],
                                 mybir.ActivationFunctionType.Ln, bias=1.0)

            # dm = data - TM (zero everywhere except anchor slots)
            dm = pool.tile([NROWS, QW], bf16)
            nc.vector.tensor_tensor(dm[:], dv, tv,
                                    op=mybir.AluOpType.subtract)
            # c = clamp(dm, -1, 1); rs = dm - c  (signed linear residual)
            cl = pool.tile([NROWS, QW], bf16)
            nc.vector.tensor_scalar(cl[:], dm[:], -1.0, 1.0,
                                    mybir.AluOpType.max,
                                    mybir.AluOpType.min)
            rs = pool.tile([NROWS, QW], bf16)
            nc.vector.tensor_tensor(rs[:], dm[:], cl[:],
                                    op=mybir.AluOpType.subtract)

            # P[0] = sum dm^2 ; P[1] = sum rs^2 (free ACT accumulators)
            sqd = pool.tile([NROWS, QW], bf16)
            nc.scalar.activation(sqd[:], dm[:],
                                 mybir.ActivationFunctionType.Square,
                                 accum_out=P[:, 0:1])
            sqr = pool.tile([NROWS, QW], bf16)
            nc.scalar.activation(sqr[:], rs[:],
                                 mybir.ActivationFunctionType.Square,
                                 accum_out=P[:, 1:2])

            nc.sync.dma_start(OUT[:, :], P[:])
    nc.compile()
    return nc


def kernel(**inputs):
    score = np.asarray(inputs["score"], dtype=np.float32)[0]            # [20,H,W]
    vp = np.asarray(inputs["vertical_pred"], dtype=np.float32)[0]
    side = np.asarray(inputs["side_refinement"], dtype=np.float32)[0]   # [10,H,W]
    pidx = np.asarray(inputs["positive"])
    nidx = np.asarray(inputs["negative"])
    vidx = np.asarray(inputs["vertical_reg_idx"])
    vtgt = np.asarray(inputs["vertical_reg_tgt"], dtype=np.float32)
    sidx = np.asarray(inputs["side_reg_idx"])
    stgt = np.asarray(inputs["side_reg_tgt"], dtype=np.float32)

    score_f = score.reshape(2 * K, HW)
    vp_f = vp.reshape(2 * K, HW)
    side_f = side.reshape(K, HW)

    if "nc" not in _cache:
        _cache["nc"] = _build_bass()
    nc = _cache["nc"]

    # ---- per-core dense layout -------------------------------------------
    # rows 0..79:   vp channel-quarter rows (r = 4*ch + q)
    # rows 80..119: side channel-quarter rows (r = 80 + 4*ch + q)
    # row 120:      cls pairs
    data = np.zeros((N_CORES, NROWS, QW), np.float32)
    data[:, 0:80, :] = vp_f.reshape(2 * K, N_CORES, 4, QW).transpose(
        1, 0, 2, 3).reshape(N_CORES, 80, QW)
    data[:, 80:120, :] = side_f.reshape(K, N_CORES, 4, QW).transpose(
        1, 0, 2, 3).reshape(N_CORES, 40, QW)

    # ---- scatter targets into TM (default = data) ------------------------
    vx = vidx[:, 0].astype(np.int64)
    vy = vidx[:, 1].astype(np.int64)
    va = vidx[:, 2].astype(np.int64)
    vpos = vy * W + vx
    vcore = vpos // PPC
    vl = vpos % PPC
    vq = vl // QW
    vu = vl % QW
    # slots for both coords
    s_core = np.concatenate([vcore, vcore])
    s_row = np.concatenate([8 * va + vq, 8 * va + 4 + vq])
    s_col = np.concatenate([vu, vu])
    s_tgt = np.concatenate([vtgt[:, 0], vtgt[:, 1]]).astype(np.float32)
    s_cls = np.zeros(len(s_core), np.int8)

    sx = sidx[:, 0].astype(np.int64)
    sy = sidx[:, 1].astype(np.int64)
    sa = sidx[:, 2].astype(np.int64)
    spos = sy * W + sx
    score_c = spos // PPC
    sl = spos % PPC
    sq_ = sl // QW
    su = sl % QW
    s_core = np.concatenate([s_core, score_c])
    s_row = np.concatenate([s_row, R_SD0 + 4 * sa + sq_])
    s_col = np.concatenate([s_col, su])
    s_tgt = np.concatenate([s_tgt, stgt]).astype(np.float32)
    s_cls = np.concatenate([s_cls, np.ones(len(sidx), np.int8)])

    tm = data.copy()
    # first occurrence of each (core,row,col) -> direct scatter
    keys = (s_core * NROWS + s_row) * QW + s_col
    _, first_i = np.unique(keys, return_index=True)
    fmask = np.zeros(len(keys), np.bool_)
    fmask[first_i] = True
    occ = np.zeros((N_CORES, NROWS, QW), np.bool_)
    occ[s_core[fmask], s_row[fmask], s_col[fmask]] = True
    tm[s_core[fmask], s_row[fmask], s_col[fmask]] = s_tgt[fmask]

    # duplicates: overwrite any anchor-free slot in a same-class row (that
    # slot contributes exactly 0 otherwise)
    CLASS_ROWS = {0: (0, 80), 1: (80, 120)}
    free_iters = {}
    for i in np.nonzero(~fmask)[0]:
        c = int(s_core[i])
        kcls = int(s_cls[i])
        val = data[c, s_row[i], s_col[i]]
        key = (c, kcls)
        if key not in free_iters:
            lo, hi = CLASS_ROWS[kcls]
            flat = np.flatnonzero(~occ[c, lo:hi, :])
            free_iters[key] = [iter(flat), lo]
        it, lo = free_iters[key]
        j = next(it)
        r, u = lo + j // QW, j % QW
        while occ[c, r, u]:
            j = next(it)
            r, u = lo + j // QW, j % QW
        occ[c, r, u] = True
        data[c, r, u] = val
        tm[c, r, u] = s_tgt[i]

    # ---- cls row: interleaved (first, second) logit pairs ----------------
    # pos: (l0, l1); neg: (l1, l0); ce = softplus(first - second)
    pa = pidx[:, 2].astype(np.int64)
    ppos = pidx[:, 1].astype(np.int64) * W + pidx[:, 0].astype(np.int64)
    na = nidx[:, 2].astype(np.int64)
    npos = nidx[:, 1].astype(np.int64) * W + nidx[:, 0].astype(np.int64)
    c_core = np.concatenate([ppos // PPC, npos // PPC])
    c_first = np.concatenate([score_f[2 * pa, ppos], score_f[2 * na + 1, npos]])
    c_second = np.concatenate([score_f[2 * pa + 1, ppos], score_f[2 * na, npos]])
    # one cls entry per partition: (first, -second); pads are (0, -40) so
    # ce = ln(1 + exp(-40)) is exactly 0 in f32
    clsf = np.zeros((N_CORES, NROWS), np.float32)
    clsn = np.full((N_CORES, NROWS), -40.0, np.float32)
    for c in range(N_CORES):
        sel = c_core == c
        n = int(sel.sum())
        assert n <= NROWS, "cls capacity exceeded"
        clsf[c, :n] = c_first[sel]
        clsn[c, :n] = -c_second[sel]

    # ---- mega staging: [data fp8 | tm fp8 | cls f32 pair] per row --------
    # junk slots have tm == data in f32, so they round to identical fp8
    # values and dm stays exactly zero
    data_q = data.astype(ml_dtypes.float8_e4m3fn)
    tm_q = tm.astype(ml_dtypes.float8_e4m3fn)
    WB = 2 * QW + 8
    in_maps = []
    for c in range(N_CORES):
        mega = np.empty((NROWS, WB), np.uint8)
        mega[:, 0:QW] = data_q[c].view(np.uint8)
        mega[:, QW:2 * QW] = tm_q[c].view(np.uint8)
        mega[:, 2 * QW:2 * QW + 4] = clsf[c].view(np.uint8).reshape(NROWS, 4)
        mega[:, 2 * QW + 4:WB] = clsn[c].view(np.uint8).reshape(NROWS, 4)
        in_maps.append({"mega": mega})

    res = bass_utils.run_bass_kernel_spmd(
        nc, in_maps, core_ids=list(range(N_CORES)))

    wv = np.zeros(NROWS, np.float32)
    wo = np.zeros(NROWS, np.float32)
    wv[0:80] = 1.0 / (2.0 * NV_REG)
    wo[80:120] = 1.0 / NO_REG
    v_loss = np.float32(0.0)
    o_loss = np.float32(0.0)
    cls_sum = np.float32(0.0)
    for c in range(N_CORES):
        P = res.results[c]["out"]      # [NROWS, 3]
        S = 0.5 * (P[:, 0] - P[:, 1])
        v_loss += np.float32(np.dot(S, wv))
        o_loss += np.float32(np.dot(S, wo))
        cls_sum += np.float32(P[:, 2].sum())
    cls_loss = np.float32(cls_sum / NS)
    loss = np.float32(cls_loss + v_loss + o_loss)
    return (np.float32(loss), np.float32(cls_loss), np.float32(v_loss),
            np.float32(o_loss))
